# revision 14
# baseline (speedup 1.0000x reference)
"""Trainium2 Bass kernel for nn_Attention_51187420234360 (sparse_attention).

Algebraic restructure of the reference (P=16384, H=256, E=64, A=64):
  q = tn @ (w_temp.T @ w_spat) + b_temp @ w_spat        [P,H]
  c = tn @ (w_temp.T @ b_spat) + b_temp @ b_spat        [P]
  g[p,m,j]    = sum_e raw[p,4m+j,e] * q[p,64j+e]        (BN scale deferred)
  attn[p,m]   = T*( sum_j scale[4m+j]*g[p,m,j] + sum_j shift[4m+j]*Q[p,j] + c[p] )
  wv          = softmax(attn over persons)
  out[p,64j+e]= sum_m raw[p,4m+j,e]*wv[p,m]*scale[4m+j] + sum_m wv[p,m]*shift[4m+j]

Big tensor read exactly twice. Persons sharded across 8 cores; 4 tiny
all-reduces (temp stats, spat stats, softmax max, softmax sum).
g-dot in f32 (softmax amplifies attn error); stats & pass-3 in bf16.
"""

import os
import sys

for _p in ("/opt/trn_rl_repo",):
    if os.path.isdir(_p) and _p not in sys.path:
        sys.path.insert(0, _p)

import numpy as np

import concourse.bass as bass
import concourse.bacc as bacc
import concourse.bass_isa as bass_isa
import concourse.mybir as mybir
from concourse import tile
from concourse.bass_utils import run_bass_kernel_spmd

F32 = mybir.dt.float32
BF16 = mybir.dt.bfloat16
AX = mybir.AxisListType
OP = mybir.AluOpType
AF = mybir.ActivationFunctionType

H = 256
E = 64
A = 64
NCORES = 8
EPS = 1e-5
P_FULL = 16384

_last_results = None  # test.py reads exec_time_ns off this
SKIP = set()  # ablation flags: 'stats', 'g', 'p3'


def build_graph(nc, PP, n_cores, p_full=P_FULL, use_cc=True):
    NT = PP // 128
    TEMPER = float(E) / float(np.sqrt(A))
    NSPAT = float(p_full * E)
    NTEMP = float(p_full)

    spat = nc.dram_tensor("spat", [PP, H * E], F32, kind="ExternalInput")
    temp = nc.dram_tensor("temp", [PP, H], F32, kind="ExternalInput")
    wqx = nc.dram_tensor("wqx", [H, 260], F32, kind="ExternalInput")
    qbx = nc.dram_tensor("qbx", [1, 260], F32, kind="ExternalInput")
    gb = nc.dram_tensor("gb", [2, H], F32, kind="ExternalInput")
    ident = nc.dram_tensor("ident", [128, 128], F32, kind="ExternalInput")
    ones = nc.dram_tensor("ones_", [128, 8], F32, kind="ExternalInput")
    out = nc.dram_tensor("out", [PP, H], F32, kind="ExternalOutput")

    rg = [list(range(n_cores))]

    def etree(pool, src_t, nseg, width, dtype, dst_ap, tagp, stop=16):
        """dst[p, nseg] = sum over innermost `width` of src[p, nseg*width]."""
        cur, w = src_t, width
        while w > stop:
            nxt = pool.tile([128, nseg * (w // 2)], dtype, tag=f"{tagp}{w//2}")
            ca = cur[:].rearrange("p (s e) -> p s e", s=nseg)
            nc.vector.tensor_add(
                nxt[:].rearrange("p (s e) -> p s e", s=nseg),
                ca[:, :, 0 : w // 2],
                ca[:, :, w // 2 : w],
            )
            cur, w = nxt, w // 2
        if w == 2:
            ca = cur[:].rearrange("p (s e) -> p s e", s=nseg)
            nc.vector.tensor_add(dst_ap.unsqueeze(2), ca[:, :, 0:1], ca[:, :, 1:2])
        else:
            nc.vector.reduce_sum(
                dst_ap, cur[:].rearrange("p (s e) -> p s e", s=nseg), axis=AX.X
            )

    with tile.TileContext(nc) as tc:
        with (
            tc.tile_pool(name="const", bufs=1) as cp,
            tc.tile_pool(name="dram", bufs=1, space="DRAM") as dp,
            tc.tile_pool(name="psum", bufs=2, space="PSUM") as psp,
            tc.tile_pool(name="small", bufs=1) as sp,
        ):
            # ---- whole-kernel constants / pass-3 persistents (~30KB/part) ----
            ident_sb = cp.tile([128, 128], F32, tag="ident")
            ones_sb = cp.tile([128, 8], F32, tag="ones")
            gb_sb = cp.tile([1, 2 * H], F32, tag="gb")
            qbx_bc = cp.tile([128, 260], F32, tag="qbx_bc")
            sc_bc = cp.tile([128, 256], F32, tag="sc_bc")
            sh_bc = cp.tile([128, 256], F32, tag="sh_bc")
            wvs2 = cp.tile([128, NT * 512], BF16, tag="wvs2")
            w_all = cp.tile([128, NT * 4], F32, tag="w_all")

            nc.sync.dma_start(out=ident_sb[:], in_=ident.ap())
            nc.sync.dma_start(out=ones_sb[:], in_=ones.ap())
            nc.sync.dma_start(out=gb_sb[:], in_=gb.ap().rearrange("a h -> (a h)").unsqueeze(0))
            qbx_1p = sp.tile([1, 260], F32, tag="qbx1p")
            nc.sync.dma_start(out=qbx_1p[:], in_=qbx.ap())
            nc.gpsimd.partition_broadcast(qbx_bc[:], qbx_1p[:])

            with tc.tile_pool(name="bpool", bufs=1) as bp:
                # ---- persistents through attn (~42KB/part) ----
                q_ext = bp.tile([128, NT * 260], F32, tag="q_ext")
                g_all = bp.tile([128, NT * 256], F32, tag="g_all")
                acc_rs = bp.tile([128, 256], F32, tag="acc_rs")
                acc_sq = bp.tile([128, 256], F32, tag="acc_sq")
                attn = bp.tile([128, NT * 64], F32, tag="attn")
                qj = bp.tile([128, NT * 4], F32, tag="qj")

                # ================= temp phase =================
                with tc.tile_pool(name="apool", bufs=1) as ap:
                    temp_sb = ap.tile([128, NT * H], F32, tag="temp_sb")
                    nc.sync.dma_start(
                        out=temp_sb[:],
                        in_=temp.ap().rearrange("(n p) h -> n p h", p=128).transpose([1, 0, 2]),
                    )
                    tsq_sb = ap.tile([128, NT * H], F32, tag="tsq_sb")
                    nc.scalar.activation(tsq_sb[:], temp_sb[:], AF.Square)
                    tacc = ap.tile([128, 2 * H], F32, tag="tacc")

                    def fold_n(dst_ap, src_t, nt):
                        cur, width = src_t, nt
                        while width > 1:
                            half = width // 2
                            ca = cur[:].rearrange("p (n h) -> p n h", n=width)
                            if half > 1:
                                nxt = ap.tile([128, half * H], F32, tag=f"fold{half}")
                                dst = nxt[:].rearrange("p (n h) -> p n h", n=half)
                            else:
                                nxt = None
                                dst = dst_ap.unsqueeze(1)
                            nc.vector.tensor_add(
                                dst, ca[:, 0:half, :], ca[:, half : 2 * half, :]
                            )
                            cur, width = nxt, half

                    fold_n(tacc[:, 0:H], temp_sb, NT)
                    fold_n(tacc[:, H : 2 * H], tsq_sb, NT)
                    ps_t = psp.tile([1, 2 * H], F32, tag="ps_t")
                    nc.tensor.matmul(
                        ps_t[:], ones_sb[:, 0:1], tacc[:], start=True, stop=True
                    )
                    ar1_sb = sp.tile([1, 2 * H], F32, tag="ar1")
                    nc.vector.tensor_copy(ar1_sb[:], ps_t[:])
                    ar1_in = dp.tile([1, 2 * H], F32, tag="ar1_in")
                    ar1_out = dp.tile([1, 2 * H], F32, tag="ar1_out")
                    nc.sync.dma_start(out=ar1_in[:], in_=ar1_sb[:])
                    (nc.gpsimd.collective_compute(
                        "AllReduce", OP.add, replica_groups=rg,
                        ins=[ar1_in[:]], outs=[ar1_out[:]],
                    ) if use_cc else nc.gpsimd.dma_start(out=ar1_out[:], in_=ar1_in[:]))
                    tstat = sp.tile([1, 2 * H], F32, tag="tstat")
                    nc.sync.dma_start(out=tstat[:], in_=ar1_out[:])

                    stt_1p = sp.tile([1, 2 * H], F32, tag="stt1p")
                    scr = sp.tile([1, H], F32, tag="scr")
                    scr2 = sp.tile([1, H], F32, tag="scr2")
                    nc.scalar.mul(scr[:], tstat[:, 0:H], 1.0 / NTEMP)
                    nc.scalar.activation(scr2[:], scr[:], AF.Square)
                    nc.vector.tensor_scalar_mul(
                        stt_1p[:, 0:H], tstat[:, H : 2 * H], 1.0 / NTEMP
                    )
                    nc.vector.tensor_sub(stt_1p[:, 0:H], stt_1p[:, 0:H], scr2[:])
                    nc.vector.tensor_scalar_add(stt_1p[:, 0:H], stt_1p[:, 0:H], EPS)
                    nc.scalar.activation(stt_1p[:, 0:H], stt_1p[:, 0:H], AF.Sqrt)
                    nc.vector.reciprocal(stt_1p[:, 0:H], stt_1p[:, 0:H])
                    nc.vector.tensor_mul(
                        stt_1p[:, 0:H], stt_1p[:, 0:H], gb_sb[:, 0:H]
                    )
                    nc.vector.tensor_mul(scr[:], scr[:], stt_1p[:, 0:H])
                    nc.vector.tensor_sub(
                        stt_1p[:, H : 2 * H], gb_sb[:, H : 2 * H], scr[:]
                    )
                    stt_bc = ap.tile([128, 2 * H], F32, tag="stt_bc")
                    nc.gpsimd.partition_broadcast(stt_bc[:], stt_1p[:])

                    # tn = temp*scale_t + shift_t
                    tn_sb = ap.tile([128, NT * H], F32, tag="tn_sb")
                    nc.vector.tensor_mul(
                        tn_sb[:].rearrange("p (n h) -> p n h", n=NT),
                        temp_sb[:].rearrange("p (n h) -> p n h", n=NT),
                        stt_bc[:, 0:H].unsqueeze(1).broadcast_to([128, NT, H]),
                    )
                    nc.vector.tensor_add(
                        tn_sb[:].rearrange("p (n h) -> p n h", n=NT),
                        tn_sb[:].rearrange("p (n h) -> p n h", n=NT),
                        stt_bc[:, H : 2 * H].unsqueeze(1).broadcast_to([128, NT, H]),
                    )
                    # q = tn @ WQx + qbx  via PE (transpose tn, then matmul)
                    wqx_sb = ap.tile([128, 2 * 260], F32, tag="wqx")
                    nc.sync.dma_start(
                        out=wqx_sb[:],
                        in_=wqx.ap().rearrange("(hh hp) n -> hh hp n", hp=128).transpose([1, 0, 2]),
                    )
                    tnT = ap.tile([128, NT * 2 * 128], F32, tag="tnT")
                    for n in range(NT):
                        for hh in range(2):
                            ps_tr = psp.tile([128, 128], F32, tag="ps_tr")
                            nc.tensor.transpose(
                                ps_tr[:],
                                tn_sb[:, n * H + hh * 128 : n * H + hh * 128 + 128],
                                ident_sb[:],
                            )
                            o = (n * 2 + hh) * 128
                            nc.vector.tensor_copy(tnT[:, o : o + 128], ps_tr[:])
                    for n in range(NT):
                        ps_q = psp.tile([128, 260], F32, tag="ps_q")
                        for hh in range(2):
                            o = (n * 2 + hh) * 128
                            nc.tensor.matmul(
                                ps_q[:],
                                tnT[:, o : o + 128],
                                wqx_sb[:, hh * 260 : hh * 260 + 260],
                                start=(hh == 0), stop=(hh == 1),
                            )
                        nc.vector.tensor_add(
                            q_ext[:, n * 260 : n * 260 + 260], ps_q[:], qbx_bc[:]
                        )
                    nc.vector.reduce_sum(
                        qj[:].rearrange("p (t j) -> p t j", t=NT),
                        q_ext[:].rearrange("p (t x) -> p t x", t=NT)[:, :, 0:256]
                        .rearrange("p t (j r) -> p t j r", j=4),
                        axis=AX.X,
                    )

                # ================= pass 1: stats + g =================
                if "g" in SKIP:
                    nc.vector.memset(g_all[:], 0.0)
                with (
                    tc.tile_pool(name="p1persist", bufs=1) as p1p,
                    tc.tile_pool(name="p1raw", bufs=2) as p1r,
                    tc.tile_pool(name="p1work", bufs=1) as p1w,
                ):
                    part_all = p1p.tile([128, NT * 256], F32, tag="part_all")
                    part2_all = p1p.tile([128, NT * 256], F32, tag="part2_all")
                    if "stats" in SKIP:
                        nc.vector.memset(part_all[:], 0.0)
                        nc.vector.memset(part2_all[:], 0.0)
                    for t in range(NT):
                        for qt in range(4):  # h-quarters of 64
                            raw = p1r.tile([128, 4096], F32, tag="raw")
                            nc.sync.dma_start(
                                out=raw[:],
                                in_=spat.ap()[
                                    t * 128 : t * 128 + 128,
                                    qt * 4096 : qt * 4096 + 4096,
                                ],
                            )
                            tmp = p1w.tile([128, 4096], F32, tag="tmp")
                            if "g" not in SKIP:
                              nc.vector.tensor_mul(
                                tmp[:].rearrange("p (m j e) -> p m j e", m=16, j=4),
                                raw[:].rearrange("p (m j e) -> p m j e", m=16, j=4),
                                q_ext[:, t * 260 : t * 260 + 256]
                                .rearrange("p (j e) -> p j e", j=4)
                                .unsqueeze(1).broadcast_to([128, 16, 4, 64]),
                            )
                            goff = t * 256 + qt * 64
                            if "g" not in SKIP:
                                etree(p1w, tmp, 64, 64, F32, g_all[:, goff : goff + 64], "trF")

                            if "stats" not in SKIP:
                                poff = t * 256 + qt * 64
                                raw_bf = p1w.tile([128, 4096], BF16, tag="cast_bf")
                                nc.scalar.activation(raw_bf[:], raw[:], AF.Copy)
                                etree(p1w, raw_bf, 64, 64, BF16,
                                      part_all[:, poff : poff + 64], "trB", stop=2)
                                sq_bf = p1w.tile([128, 4096], BF16, tag="cast_bf")
                                nc.scalar.activation(sq_bf[:], raw[:], AF.Square)
                                etree(p1w, sq_bf, 64, 64, BF16,
                                      part2_all[:, poff : poff + 64], "trB", stop=2)

                    # fold per-(t,qt) partials over t into acc (in place)
                    for (pa, acc) in ((part_all, acc_rs), (part2_all, acc_sq)):
                        w = NT
                        while w > 1:
                            ca = pa[:].rearrange("p (t x) -> p t x", t=NT)
                            nc.vector.tensor_add(
                                ca[:, 0 : w // 2, :], ca[:, 0 : w // 2, :],
                                ca[:, w // 2 : w, :],
                            )
                            w //= 2
                        nc.vector.tensor_copy(acc[:], pa[:, 0:256])

                # ---- spat stats AR + scale/shift ----
                ps_s = psp.tile([1, 2 * H], F32, tag="ps_s")
                nc.tensor.matmul(
                    ps_s[:, 0:H], ones_sb[:, 0:1], acc_rs[:], start=True, stop=True
                )
                nc.tensor.matmul(
                    ps_s[:, H : 2 * H], ones_sb[:, 0:1], acc_sq[:], start=True, stop=True
                )
                ar2_sb = sp.tile([1, 2 * H], F32, tag="ar2")
                nc.vector.tensor_copy(ar2_sb[:], ps_s[:])
                ar2_in = dp.tile([1, 2 * H], F32, tag="ar2_in")
                ar2_out = dp.tile([1, 2 * H], F32, tag="ar2_out")
                nc.sync.dma_start(out=ar2_in[:], in_=ar2_sb[:])
                (nc.gpsimd.collective_compute(
                    "AllReduce", OP.add, replica_groups=rg,
                    ins=[ar2_in[:]], outs=[ar2_out[:]],
                ) if use_cc else nc.gpsimd.dma_start(out=ar2_out[:], in_=ar2_in[:]))
                sstat = sp.tile([1, 2 * H], F32, tag="sstat")
                nc.sync.dma_start(out=sstat[:], in_=ar2_out[:])

                ss_1p = sp.tile([1, 2 * H], F32, tag="ss1p")
                scrb = sp.tile([1, H], F32, tag="scrb")
                scrb2 = sp.tile([1, H], F32, tag="scrb2")
                nc.scalar.mul(scrb[:], sstat[:, 0:H], 1.0 / NSPAT)
                nc.scalar.activation(scrb2[:], scrb[:], AF.Square)
                nc.vector.tensor_scalar_mul(
                    ss_1p[:, 0:H], sstat[:, H : 2 * H], 1.0 / NSPAT
                )
                nc.vector.tensor_sub(ss_1p[:, 0:H], ss_1p[:, 0:H], scrb2[:])
                nc.vector.tensor_scalar_add(ss_1p[:, 0:H], ss_1p[:, 0:H], EPS)
                nc.scalar.activation(ss_1p[:, 0:H], ss_1p[:, 0:H], AF.Sqrt)
                nc.vector.reciprocal(ss_1p[:, 0:H], ss_1p[:, 0:H])
                nc.vector.tensor_mul(ss_1p[:, 0:H], ss_1p[:, 0:H], gb_sb[:, 0:H])
                nc.vector.tensor_mul(scrb[:], scrb[:], ss_1p[:, 0:H])
                nc.vector.tensor_sub(
                    ss_1p[:, H : 2 * H], gb_sb[:, H : 2 * H], scrb[:]
                )
                nc.gpsimd.partition_broadcast(sc_bc[:], ss_1p[:, 0:H])
                nc.gpsimd.partition_broadcast(sh_bc[:], ss_1p[:, H : 2 * H])

                # ================= attn + softmax =================
                with tc.tile_pool(name="atpool", bufs=1) as atp:
                    gtmp = atp.tile([128, NT * 256], F32, tag="gtmp")
                    nc.vector.tensor_mul(
                        gtmp[:].rearrange("p (t x) -> p t x", t=NT),
                        g_all[:].rearrange("p (t x) -> p t x", t=NT),
                        sc_bc[:].unsqueeze(1).broadcast_to([128, NT, 256]),
                    )
                    nc.vector.reduce_sum(
                        attn[:].rearrange("p (t m) -> p t m", t=NT),
                        gtmp[:].rearrange("p (t m j) -> p t m j", t=NT, m=64),
                        axis=AX.X,
                    )
                    nc.vector.tensor_mul(
                        gtmp[:].rearrange("p (t m j) -> p t m j", t=NT, m=64),
                        qj[:].rearrange("p (t j) -> p t j", t=NT)
                        .unsqueeze(2).broadcast_to([128, NT, 64, 4]),
                        sh_bc[:].rearrange("p (m j) -> p m j", m=64)
                        .unsqueeze(1).broadcast_to([128, NT, 64, 4]),
                    )
                    a2 = atp.tile([128, NT * 64], F32, tag="a2")
                    nc.vector.reduce_sum(
                        a2[:].rearrange("p (t m) -> p t m", t=NT),
                        gtmp[:].rearrange("p (t m j) -> p t m j", t=NT, m=64),
                        axis=AX.X,
                    )
                    nc.vector.tensor_add(attn[:], attn[:], a2[:])
                    nc.vector.tensor_add(
                        attn[:].rearrange("p (t m) -> p t m", t=NT),
                        attn[:].rearrange("p (t m) -> p t m", t=NT),
                        q_ext[:].rearrange("p (t x) -> p t x", t=NT)[:, :, 256:257]
                        .broadcast_to([128, NT, 64]),
                    )
                    nc.vector.tensor_scalar_mul(attn[:], attn[:], TEMPER)

                    red = sp.tile([128, 64], F32, tag="red")
                    redr = sp.tile([128, 64], F32, tag="redr")
                    nc.vector.reduce_max(
                        red[:], attn[:].rearrange("p (t m) -> p m t", t=NT), axis=AX.X
                    )
                    nc.gpsimd.partition_all_reduce(
                        redr[:], red[:], channels=128,
                        reduce_op=bass_isa.ReduceOp.max,
                    )
                    ar3_in = dp.tile([1, 64], F32, tag="ar3_in")
                    ar3_out = dp.tile([1, 64], F32, tag="ar3_out")
                    nc.sync.dma_start(out=ar3_in[:], in_=redr[0:1, :])
                    (nc.gpsimd.collective_compute(
                        "AllReduce", OP.max, replica_groups=rg,
                        ins=[ar3_in[:]], outs=[ar3_out[:]],
                    ) if use_cc else nc.gpsimd.dma_start(out=ar3_out[:], in_=ar3_in[:]))
                    mx1 = sp.tile([1, 64], F32, tag="mx1")
                    nc.sync.dma_start(out=mx1[:], in_=ar3_out[:])
                    mx_bc = sp.tile([128, 64], F32, tag="mx_bc")
                    nc.gpsimd.partition_broadcast(mx_bc[:], mx1[:])

                    nc.vector.tensor_sub(
                        attn[:].rearrange("p (t m) -> p t m", t=NT),
                        attn[:].rearrange("p (t m) -> p t m", t=NT),
                        mx_bc[:].unsqueeze(1).broadcast_to([128, NT, 64]),
                    )
                    nc.scalar.activation(attn[:], attn[:], AF.Exp)
                    nc.vector.reduce_sum(
                        red[:], attn[:].rearrange("p (t m) -> p m t", t=NT), axis=AX.X
                    )
                    nc.gpsimd.partition_all_reduce(
                        redr[:], red[:], channels=128,
                        reduce_op=bass_isa.ReduceOp.add,
                    )
                    ar4_in = dp.tile([1, 64], F32, tag="ar4_in")
                    ar4_out = dp.tile([1, 64], F32, tag="ar4_out")
                    nc.sync.dma_start(out=ar4_in[:], in_=redr[0:1, :])
                    (nc.gpsimd.collective_compute(
                        "AllReduce", OP.add, replica_groups=rg,
                        ins=[ar4_in[:]], outs=[ar4_out[:]],
                    ) if use_cc else nc.gpsimd.dma_start(out=ar4_out[:], in_=ar4_in[:]))
                    s1 = sp.tile([1, 64], F32, tag="s1")
                    nc.sync.dma_start(out=s1[:], in_=ar4_out[:])
                    nc.vector.reciprocal(s1[:], s1[:])
                    rs_bc = sp.tile([128, 64], F32, tag="rs_bc")
                    nc.gpsimd.partition_broadcast(rs_bc[:], s1[:])
                    nc.vector.tensor_mul(
                        attn[:].rearrange("p (t m) -> p t m", t=NT),
                        attn[:].rearrange("p (t m) -> p t m", t=NT),
                        rs_bc[:].unsqueeze(1).broadcast_to([128, NT, 64]),
                    )

                    wvs = atp.tile([128, NT * 256], BF16, tag="wvs")
                    nc.vector.tensor_mul(
                        wvs[:].rearrange("p (t m j) -> p t m j", t=NT, m=64),
                        attn[:].rearrange("p (t m) -> p t m", t=NT)
                        .unsqueeze(3).broadcast_to([128, NT, 64, 4]),
                        sc_bc[:].rearrange("p (m j) -> p m j", m=64)
                        .unsqueeze(1).broadcast_to([128, NT, 64, 4]),
                    )
                    nc.vector.tensor_copy(
                        wvs2[:].rearrange("p (x d) -> p x d", d=2),
                        wvs[:].unsqueeze(2).broadcast_to([128, NT * 256, 2]),
                    )
                    nc.vector.tensor_mul(
                        gtmp[:].rearrange("p (t m j) -> p t m j", t=NT, m=64),
                        attn[:].rearrange("p (t m) -> p t m", t=NT)
                        .unsqueeze(3).broadcast_to([128, NT, 64, 4]),
                        sh_bc[:].rearrange("p (m j) -> p m j", m=64)
                        .unsqueeze(1).broadcast_to([128, NT, 64, 4]),
                    )
                    nc.vector.reduce_sum(
                        w_all[:].rearrange("p (t j) -> p t j", t=NT),
                        gtmp[:].rearrange("p (t m j) -> p t j m", t=NT, m=64),
                        axis=AX.X,
                    )

            # ================= pass 3: output =================
            with (
                tc.tile_pool(name="p3raw", bufs=2) as p3r,
                tc.tile_pool(name="p3work", bufs=1) as p3w,
            ):
                for t in range(NT):
                    halfpart = p3w.tile([128, 2 * 256], F32, tag="halfpart")
                    for hf in range(2):
                        raw3 = p3r.tile([128, 8192], F32, tag="raw3")
                        nc.sync.dma_start(
                            out=raw3[:],
                            in_=spat.ap()[
                                t * 128 : t * 128 + 128,
                                hf * 8192 : hf * 8192 + 8192,
                            ],
                        )
                        raw_b = p3w.tile([128, 8192], BF16, tag="rawb3")
                        nc.scalar.activation(raw_b[:], raw3[:], AF.Copy)
                        if "p3c" in SKIP:
                            nc.vector.tensor_copy(
                                halfpart[:, hf * 256 : hf * 256 + 256],
                                raw_b[:, 0:256])
                            continue
                        woff = t * 512 + hf * 256
                        nc.vector.tensor_mul(
                            raw_b[:].rearrange("p (h e2 d) -> p h e2 d", h=128, e2=32),
                            raw_b[:].rearrange("p (h e2 d) -> p h e2 d", h=128, e2=32),
                            wvs2[:, woff : woff + 256]
                            .rearrange("p (h d) -> p h d", h=128)
                            .unsqueeze(2).broadcast_to([128, 128, 32, 2]),
                        )
                        # tree over m within half: m_local 32 -> 4, then reduce
                        cur, width = raw_b, 32
                        while width > 4:
                            nxt = p3w.tile(
                                [128, (width // 2) * 256], BF16, tag=f"t3_{width//2}"
                            )
                            ca = cur[:].rearrange("p (m x) -> p m x", m=width)
                            nc.vector.tensor_add(
                                nxt[:].rearrange("p (m x) -> p m x", m=width // 2),
                                ca[:, 0 : width // 2, :],
                                ca[:, width // 2 : width, :],
                            )
                            cur, width = nxt, width // 2
                        nc.vector.reduce_sum(
                            halfpart[:, hf * 256 : hf * 256 + 256]
                            .rearrange("p (j e) -> p j e", j=4),
                            cur[:].rearrange("p (m j e) -> p j e m", m=4, j=4),
                            axis=AX.X,
                        )
                    out_t = p3w.tile([128, 256], F32, tag="out_t")
                    nc.vector.tensor_add(
                        out_t[:], halfpart[:, 0:256], halfpart[:, 256:512]
                    )
                    nc.vector.tensor_add(
                        out_t[:].rearrange("p (j e) -> p j e", j=4),
                        out_t[:].rearrange("p (j e) -> p j e", j=4),
                        w_all[:, t * 4 : t * 4 + 4]
                        .unsqueeze(2).broadcast_to([128, 4, 64]),
                    )
                    nc.sync.dma_start(
                        out=out.ap()[t * 128 : t * 128 + 128, :], in_=out_t[:]
                    )
    return nc


def _prep_inputs(temp_hidden, spat_hidden, bn_gamma, bn_beta, w_temp, b_temp,
                 w_spat, b_spat, PP, n_cores):
    wq = (w_temp.T.astype(np.float64) @ w_spat.astype(np.float64)).astype(np.float32)
    wc = (w_temp.T @ b_spat).astype(np.float32)
    qb0 = (b_temp @ w_spat).astype(np.float32)
    cc0 = np.float32(b_temp @ b_spat)
    wqx = np.zeros((H, 260), np.float32)
    wqx[:, 0:H] = wq
    wqx[:, 256] = wc
    qbx = np.zeros((1, 260), np.float32)
    qbx[0, 0:H] = qb0
    qbx[0, 256] = cc0
    gb = np.stack([bn_gamma, bn_beta]).astype(np.float32)
    ident = np.eye(128, dtype=np.float32)
    ones_ = np.ones((128, 8), np.float32)

    in_maps = []
    for i in range(n_cores):
        sl = slice(i * PP, (i + 1) * PP)
        in_maps.append({
            "spat": np.ascontiguousarray(
                spat_hidden[sl].reshape(PP, H * E)).astype(np.float32),
            "temp": np.ascontiguousarray(temp_hidden[sl]).astype(np.float32),
            "wqx": wqx, "qbx": qbx, "gb": gb, "ident": ident, "ones_": ones_,
        })
    return in_maps


def kernel(temp_hidden, spat_hidden, bn_gamma, bn_beta, w_temp, b_temp,
           w_spat, b_spat):
    global _last_results
    temp_hidden = np.asarray(temp_hidden, dtype=np.float32)
    spat_hidden = np.asarray(spat_hidden, dtype=np.float32)
    P = temp_hidden.shape[0]
    PP = P // NCORES
    in_maps = _prep_inputs(
        temp_hidden, spat_hidden,
        np.asarray(bn_gamma, dtype=np.float32), np.asarray(bn_beta, dtype=np.float32),
        np.asarray(w_temp, dtype=np.float32), np.asarray(b_temp, dtype=np.float32),
        np.asarray(w_spat, dtype=np.float32), np.asarray(b_spat, dtype=np.float32),
        PP, NCORES)

    nc = bacc.Bacc("TRN2", target_bir_lowering=False, debug=False,
                   num_devices=NCORES)
    build_graph(nc, PP, NCORES, p_full=P)
    nc.compile()
    res = run_bass_kernel_spmd(nc, in_maps, core_ids=list(range(NCORES)))
    _last_results = res
    out = np.concatenate([res.results[i]["out"] for i in range(NCORES)], axis=0)
    return out.astype(np.float32)


# revision 15
# speedup vs baseline: 1.5517x; 1.5517x over previous
"""Trainium2 Bass kernel for nn_Attention_51187420234360 (sparse_attention).

Algebraic restructure of the reference (P=16384, H=256, E=64, A=64):
  q = tn @ (w_temp.T @ w_spat) + b_temp @ w_spat        [P,H]
  c = tn @ (w_temp.T @ b_spat) + b_temp @ b_spat        [P]
  g[p,m,j]    = sum_e raw[p,4m+j,e] * q[p,64j+e]        (BN scale deferred)
  attn[p,m]   = T*( sum_j scale[4m+j]*g[p,m,j] + sum_j shift[4m+j]*Q[p,j] + c[p] )
  wv          = softmax(attn over persons)
  out[p,64j+e]= sum_m raw[p,4m+j,e]*wv[p,m]*scale[4m+j] + sum_m wv[p,m]*shift[4m+j]

Big tensor read exactly twice. Persons sharded across 8 cores; 4 tiny
all-reduces (temp stats, spat stats, softmax max, softmax sum).
g-dot in f32 (softmax amplifies attn error); stats & pass-3 in bf16.
"""

import os
import sys

for _p in ("/opt/trn_rl_repo",):
    if os.path.isdir(_p) and _p not in sys.path:
        sys.path.insert(0, _p)

import numpy as np

import concourse.bass as bass
import concourse.bacc as bacc
import concourse.bass_isa as bass_isa
import concourse.mybir as mybir
from concourse import tile
from concourse.bass_utils import run_bass_kernel_spmd

F32 = mybir.dt.float32
BF16 = mybir.dt.bfloat16
AX = mybir.AxisListType
OP = mybir.AluOpType
AF = mybir.ActivationFunctionType

H = 256
E = 64
A = 64
NCORES = 8
EPS = 1e-5
P_FULL = 16384

_last_results = None  # test.py reads exec_time_ns off this
SKIP = set()  # ablation flags: 'stats', 'g', 'p3'


def build_graph(nc, PP, n_cores, p_full=P_FULL, use_cc=True):
    NT = PP // 128
    TEMPER = float(E) / float(np.sqrt(A))
    NSPAT = float(p_full * E)
    NTEMP = float(p_full)

    spat = nc.dram_tensor("spat", [PP, H * E], F32, kind="ExternalInput")
    temp = nc.dram_tensor("temp", [PP, H], F32, kind="ExternalInput")
    wqx = nc.dram_tensor("wqx", [H, 260], F32, kind="ExternalInput")
    qbx = nc.dram_tensor("qbx", [1, 260], F32, kind="ExternalInput")
    gb = nc.dram_tensor("gb", [2, H], F32, kind="ExternalInput")
    ident = nc.dram_tensor("ident", [128, 128], F32, kind="ExternalInput")
    ones = nc.dram_tensor("ones_", [128, 8], F32, kind="ExternalInput")
    out = nc.dram_tensor("out", [PP, H], F32, kind="ExternalOutput")

    rg = [list(range(n_cores))]

    def etree(pool, src_t, nseg, width, dtype, dst_ap, tagp, stop=16):
        """dst[p, nseg] = sum over innermost `width` of src[p, nseg*width]."""
        cur, w = src_t, width
        while w > stop:
            nxt = pool.tile([128, nseg * (w // 2)], dtype, tag=f"{tagp}{w//2}")
            ca = cur[:].rearrange("p (s e) -> p s e", s=nseg)
            nc.vector.tensor_add(
                nxt[:].rearrange("p (s e) -> p s e", s=nseg),
                ca[:, :, 0 : w // 2],
                ca[:, :, w // 2 : w],
            )
            cur, w = nxt, w // 2
        if w == 2:
            ca = cur[:].rearrange("p (s e) -> p s e", s=nseg)
            nc.vector.tensor_add(dst_ap.unsqueeze(2), ca[:, :, 0:1], ca[:, :, 1:2])
        else:
            nc.vector.reduce_sum(
                dst_ap, cur[:].rearrange("p (s e) -> p s e", s=nseg), axis=AX.X
            )

    with tile.TileContext(nc) as tc:
        with (
            tc.tile_pool(name="const", bufs=1) as cp,
            tc.tile_pool(name="dram", bufs=1, space="DRAM") as dp,
            tc.tile_pool(name="psum", bufs=2, space="PSUM") as psp,
            tc.tile_pool(name="small", bufs=1) as sp,
        ):
            # ---- whole-kernel constants / pass-3 persistents (~30KB/part) ----
            ident_sb = cp.tile([128, 128], F32, tag="ident")
            ones_sb = cp.tile([128, 8], F32, tag="ones")
            gb_sb = cp.tile([1, 2 * H], F32, tag="gb")
            qbx_bc = cp.tile([128, 260], F32, tag="qbx_bc")
            sc_bc = cp.tile([128, 256], F32, tag="sc_bc")
            sh_bc = cp.tile([128, 256], F32, tag="sh_bc")
            wvs2 = cp.tile([128, NT * 512], BF16, tag="wvs2")
            w_all = cp.tile([128, NT * 4], F32, tag="w_all")

            nc.sync.dma_start(out=ident_sb[:], in_=ident.ap())
            nc.sync.dma_start(out=ones_sb[:], in_=ones.ap())
            nc.sync.dma_start(out=gb_sb[:], in_=gb.ap().rearrange("a h -> (a h)").unsqueeze(0))
            qbx_1p = sp.tile([1, 260], F32, tag="qbx1p")
            nc.sync.dma_start(out=qbx_1p[:], in_=qbx.ap())
            nc.gpsimd.partition_broadcast(qbx_bc[:], qbx_1p[:])

            with tc.tile_pool(name="bpool", bufs=1) as bp:
                # ---- persistents through attn (~42KB/part) ----
                q_ext = bp.tile([128, NT * 260], F32, tag="q_ext")
                g_all = bp.tile([128, NT * 256], F32, tag="g_all")
                acc_rs = bp.tile([128, 256], F32, tag="acc_rs")
                acc_sq = bp.tile([128, 256], F32, tag="acc_sq")
                attn = bp.tile([128, NT * 64], F32, tag="attn")
                qj = bp.tile([128, NT * 4], F32, tag="qj")

                # ================= temp phase =================
                with tc.tile_pool(name="apool", bufs=1) as ap:
                    temp_sb = ap.tile([128, NT * H], F32, tag="temp_sb")
                    nc.sync.dma_start(
                        out=temp_sb[:],
                        in_=temp.ap().rearrange("(n p) h -> n p h", p=128).transpose([1, 0, 2]),
                    )
                    tsq_sb = ap.tile([128, NT * H], F32, tag="tsq_sb")
                    nc.scalar.activation(tsq_sb[:], temp_sb[:], AF.Square)
                    tacc = ap.tile([128, 2 * H], F32, tag="tacc")

                    def fold_n(dst_ap, src_t, nt):
                        cur, width = src_t, nt
                        while width > 1:
                            half = width // 2
                            ca = cur[:].rearrange("p (n h) -> p n h", n=width)
                            if half > 1:
                                nxt = ap.tile([128, half * H], F32, tag=f"fold{half}")
                                dst = nxt[:].rearrange("p (n h) -> p n h", n=half)
                            else:
                                nxt = None
                                dst = dst_ap.unsqueeze(1)
                            nc.vector.tensor_add(
                                dst, ca[:, 0:half, :], ca[:, half : 2 * half, :]
                            )
                            cur, width = nxt, half

                    fold_n(tacc[:, 0:H], temp_sb, NT)
                    fold_n(tacc[:, H : 2 * H], tsq_sb, NT)
                    ps_t = psp.tile([1, 2 * H], F32, tag="ps_t")
                    nc.tensor.matmul(
                        ps_t[:], ones_sb[:, 0:1], tacc[:], start=True, stop=True
                    )
                    ar1_sb = sp.tile([1, 2 * H], F32, tag="ar1")
                    nc.vector.tensor_copy(ar1_sb[:], ps_t[:])
                    ar1_in = dp.tile([1, 2 * H], F32, tag="ar1_in")
                    ar1_out = dp.tile([1, 2 * H], F32, tag="ar1_out")
                    nc.sync.dma_start(out=ar1_in[:], in_=ar1_sb[:])
                    (nc.gpsimd.collective_compute(
                        "AllReduce", OP.add, replica_groups=rg,
                        ins=[ar1_in[:]], outs=[ar1_out[:]],
                    ) if use_cc else nc.gpsimd.dma_start(out=ar1_out[:], in_=ar1_in[:]))
                    tstat = sp.tile([1, 2 * H], F32, tag="tstat")
                    nc.sync.dma_start(out=tstat[:], in_=ar1_out[:])

                    stt_1p = sp.tile([1, 2 * H], F32, tag="stt1p")
                    scr = sp.tile([1, H], F32, tag="scr")
                    scr2 = sp.tile([1, H], F32, tag="scr2")
                    nc.scalar.mul(scr[:], tstat[:, 0:H], 1.0 / NTEMP)
                    nc.scalar.activation(scr2[:], scr[:], AF.Square)
                    nc.vector.tensor_scalar_mul(
                        stt_1p[:, 0:H], tstat[:, H : 2 * H], 1.0 / NTEMP
                    )
                    nc.vector.tensor_sub(stt_1p[:, 0:H], stt_1p[:, 0:H], scr2[:])
                    nc.vector.tensor_scalar_add(stt_1p[:, 0:H], stt_1p[:, 0:H], EPS)
                    nc.scalar.activation(stt_1p[:, 0:H], stt_1p[:, 0:H], AF.Sqrt)
                    nc.vector.reciprocal(stt_1p[:, 0:H], stt_1p[:, 0:H])
                    nc.vector.tensor_mul(
                        stt_1p[:, 0:H], stt_1p[:, 0:H], gb_sb[:, 0:H]
                    )
                    nc.vector.tensor_mul(scr[:], scr[:], stt_1p[:, 0:H])
                    nc.vector.tensor_sub(
                        stt_1p[:, H : 2 * H], gb_sb[:, H : 2 * H], scr[:]
                    )
                    stt_bc = ap.tile([128, 2 * H], F32, tag="stt_bc")
                    nc.gpsimd.partition_broadcast(stt_bc[:], stt_1p[:])

                    # tn = temp*scale_t + shift_t
                    tn_sb = ap.tile([128, NT * H], F32, tag="tn_sb")
                    nc.vector.tensor_mul(
                        tn_sb[:].rearrange("p (n h) -> p n h", n=NT),
                        temp_sb[:].rearrange("p (n h) -> p n h", n=NT),
                        stt_bc[:, 0:H].unsqueeze(1).broadcast_to([128, NT, H]),
                    )
                    nc.vector.tensor_add(
                        tn_sb[:].rearrange("p (n h) -> p n h", n=NT),
                        tn_sb[:].rearrange("p (n h) -> p n h", n=NT),
                        stt_bc[:, H : 2 * H].unsqueeze(1).broadcast_to([128, NT, H]),
                    )
                    # q = tn @ WQx + qbx  via PE (transpose tn, then matmul)
                    wqx_sb = ap.tile([128, 2 * 260], F32, tag="wqx")
                    nc.sync.dma_start(
                        out=wqx_sb[:],
                        in_=wqx.ap().rearrange("(hh hp) n -> hh hp n", hp=128).transpose([1, 0, 2]),
                    )
                    tnT = ap.tile([128, NT * 2 * 128], F32, tag="tnT")
                    for n in range(NT):
                        for hh in range(2):
                            ps_tr = psp.tile([128, 128], F32, tag="ps_tr")
                            nc.tensor.transpose(
                                ps_tr[:],
                                tn_sb[:, n * H + hh * 128 : n * H + hh * 128 + 128],
                                ident_sb[:],
                            )
                            o = (n * 2 + hh) * 128
                            nc.vector.tensor_copy(tnT[:, o : o + 128], ps_tr[:])
                    for n in range(NT):
                        ps_q = psp.tile([128, 260], F32, tag="ps_q")
                        for hh in range(2):
                            o = (n * 2 + hh) * 128
                            nc.tensor.matmul(
                                ps_q[:],
                                tnT[:, o : o + 128],
                                wqx_sb[:, hh * 260 : hh * 260 + 260],
                                start=(hh == 0), stop=(hh == 1),
                            )
                        nc.vector.tensor_add(
                            q_ext[:, n * 260 : n * 260 + 260], ps_q[:], qbx_bc[:]
                        )
                    nc.vector.reduce_sum(
                        qj[:].rearrange("p (t j) -> p t j", t=NT),
                        q_ext[:].rearrange("p (t x) -> p t x", t=NT)[:, :, 0:256]
                        .rearrange("p t (j r) -> p t j r", j=4),
                        axis=AX.X,
                    )

                # ================= pass 1: stats + g =================
                if "g" in SKIP:
                    nc.vector.memset(g_all[:], 0.0)
                with (
                    tc.tile_pool(name="p1persist", bufs=1) as p1p,
                    tc.tile_pool(name="p1raw", bufs=2) as p1r,
                    tc.tile_pool(name="p1work", bufs=1) as p1w,
                ):
                    part_all = p1p.tile([128, NT * 256], F32, tag="part_all")
                    part2_all = p1p.tile([128, NT * 256], F32, tag="part2_all")
                    if "stats" in SKIP:
                        nc.vector.memset(part_all[:], 0.0)
                        nc.vector.memset(part2_all[:], 0.0)
                    for t in range(NT):
                        for qt in range(4):  # h-quarters of 64
                            raw = p1r.tile([128, 4096], F32, tag="raw")
                            nc.sync.dma_start(
                                out=raw[:],
                                in_=spat.ap()[
                                    t * 128 : t * 128 + 128,
                                    qt * 4096 : qt * 4096 + 4096,
                                ],
                            )
                            tmp = p1w.tile([128, 4096], F32, tag="tmp")
                            if "g" not in SKIP:
                              nc.vector.tensor_mul(
                                tmp[:].rearrange("p (m j e) -> p m j e", m=16, j=4),
                                raw[:].rearrange("p (m j e) -> p m j e", m=16, j=4),
                                q_ext[:, t * 260 : t * 260 + 256]
                                .rearrange("p (j e) -> p j e", j=4)
                                .unsqueeze(1).broadcast_to([128, 16, 4, 64]),
                            )
                            goff = t * 256 + qt * 64
                            if "g" not in SKIP:
                                etree(p1w, tmp, 64, 64, F32, g_all[:, goff : goff + 64], "trF")

                            if "stats" not in SKIP:
                                poff = t * 256 + qt * 64
                                raw_bf = p1w.tile([128, 4096], BF16, tag="cast_bf", bufs=2)
                                nc.scalar.activation(raw_bf[:], raw[:], AF.Copy)
                                etree(p1w, raw_bf, 64, 64, BF16,
                                      part_all[:, poff : poff + 64], "trB", stop=2)
                                sq_bf = p1w.tile([128, 4096], BF16, tag="cast_bf", bufs=2)
                                nc.scalar.activation(sq_bf[:], raw[:], AF.Square)
                                etree(p1w, sq_bf, 64, 64, BF16,
                                      part2_all[:, poff : poff + 64], "trB", stop=2)

                    # fold per-(t,qt) partials over t into acc (in place)
                    for (pa, acc) in ((part_all, acc_rs), (part2_all, acc_sq)):
                        w = NT
                        while w > 1:
                            ca = pa[:].rearrange("p (t x) -> p t x", t=NT)
                            nc.vector.tensor_add(
                                ca[:, 0 : w // 2, :], ca[:, 0 : w // 2, :],
                                ca[:, w // 2 : w, :],
                            )
                            w //= 2
                        nc.vector.tensor_copy(acc[:], pa[:, 0:256])

                # ---- spat stats AR + scale/shift ----
                ps_s = psp.tile([1, 2 * H], F32, tag="ps_s")
                nc.tensor.matmul(
                    ps_s[:, 0:H], ones_sb[:, 0:1], acc_rs[:], start=True, stop=True
                )
                nc.tensor.matmul(
                    ps_s[:, H : 2 * H], ones_sb[:, 0:1], acc_sq[:], start=True, stop=True
                )
                ar2_sb = sp.tile([1, 2 * H], F32, tag="ar2")
                nc.vector.tensor_copy(ar2_sb[:], ps_s[:])
                ar2_in = dp.tile([1, 2 * H], F32, tag="ar2_in")
                ar2_out = dp.tile([1, 2 * H], F32, tag="ar2_out")
                nc.sync.dma_start(out=ar2_in[:], in_=ar2_sb[:])
                (nc.gpsimd.collective_compute(
                    "AllReduce", OP.add, replica_groups=rg,
                    ins=[ar2_in[:]], outs=[ar2_out[:]],
                ) if use_cc else nc.gpsimd.dma_start(out=ar2_out[:], in_=ar2_in[:]))
                sstat = sp.tile([1, 2 * H], F32, tag="sstat")
                nc.sync.dma_start(out=sstat[:], in_=ar2_out[:])

                ss_1p = sp.tile([1, 2 * H], F32, tag="ss1p")
                scrb = sp.tile([1, H], F32, tag="scrb")
                scrb2 = sp.tile([1, H], F32, tag="scrb2")
                nc.scalar.mul(scrb[:], sstat[:, 0:H], 1.0 / NSPAT)
                nc.scalar.activation(scrb2[:], scrb[:], AF.Square)
                nc.vector.tensor_scalar_mul(
                    ss_1p[:, 0:H], sstat[:, H : 2 * H], 1.0 / NSPAT
                )
                nc.vector.tensor_sub(ss_1p[:, 0:H], ss_1p[:, 0:H], scrb2[:])
                nc.vector.tensor_scalar_add(ss_1p[:, 0:H], ss_1p[:, 0:H], EPS)
                nc.scalar.activation(ss_1p[:, 0:H], ss_1p[:, 0:H], AF.Sqrt)
                nc.vector.reciprocal(ss_1p[:, 0:H], ss_1p[:, 0:H])
                nc.vector.tensor_mul(ss_1p[:, 0:H], ss_1p[:, 0:H], gb_sb[:, 0:H])
                nc.vector.tensor_mul(scrb[:], scrb[:], ss_1p[:, 0:H])
                nc.vector.tensor_sub(
                    ss_1p[:, H : 2 * H], gb_sb[:, H : 2 * H], scrb[:]
                )
                nc.gpsimd.partition_broadcast(sc_bc[:], ss_1p[:, 0:H])
                nc.gpsimd.partition_broadcast(sh_bc[:], ss_1p[:, H : 2 * H])

                # ================= attn + softmax =================
                with tc.tile_pool(name="atpool", bufs=1) as atp:
                    gtmp = atp.tile([128, NT * 256], F32, tag="gtmp")
                    nc.vector.tensor_mul(
                        gtmp[:].rearrange("p (t x) -> p t x", t=NT),
                        g_all[:].rearrange("p (t x) -> p t x", t=NT),
                        sc_bc[:].unsqueeze(1).broadcast_to([128, NT, 256]),
                    )
                    nc.vector.reduce_sum(
                        attn[:].rearrange("p (t m) -> p t m", t=NT),
                        gtmp[:].rearrange("p (t m j) -> p t m j", t=NT, m=64),
                        axis=AX.X,
                    )
                    nc.vector.tensor_mul(
                        gtmp[:].rearrange("p (t m j) -> p t m j", t=NT, m=64),
                        qj[:].rearrange("p (t j) -> p t j", t=NT)
                        .unsqueeze(2).broadcast_to([128, NT, 64, 4]),
                        sh_bc[:].rearrange("p (m j) -> p m j", m=64)
                        .unsqueeze(1).broadcast_to([128, NT, 64, 4]),
                    )
                    a2 = atp.tile([128, NT * 64], F32, tag="a2")
                    nc.vector.reduce_sum(
                        a2[:].rearrange("p (t m) -> p t m", t=NT),
                        gtmp[:].rearrange("p (t m j) -> p t m j", t=NT, m=64),
                        axis=AX.X,
                    )
                    nc.vector.tensor_add(attn[:], attn[:], a2[:])
                    nc.vector.tensor_add(
                        attn[:].rearrange("p (t m) -> p t m", t=NT),
                        attn[:].rearrange("p (t m) -> p t m", t=NT),
                        q_ext[:].rearrange("p (t x) -> p t x", t=NT)[:, :, 256:257]
                        .broadcast_to([128, NT, 64]),
                    )
                    nc.vector.tensor_scalar_mul(attn[:], attn[:], TEMPER)

                    red = sp.tile([128, 64], F32, tag="red")
                    redr = sp.tile([128, 64], F32, tag="redr")
                    nc.vector.reduce_max(
                        red[:], attn[:].rearrange("p (t m) -> p m t", t=NT), axis=AX.X
                    )
                    nc.gpsimd.partition_all_reduce(
                        redr[:], red[:], channels=128,
                        reduce_op=bass_isa.ReduceOp.max,
                    )
                    ar3_in = dp.tile([1, 64], F32, tag="ar3_in")
                    ar3_out = dp.tile([1, 64], F32, tag="ar3_out")
                    nc.sync.dma_start(out=ar3_in[:], in_=redr[0:1, :])
                    (nc.gpsimd.collective_compute(
                        "AllReduce", OP.max, replica_groups=rg,
                        ins=[ar3_in[:]], outs=[ar3_out[:]],
                    ) if use_cc else nc.gpsimd.dma_start(out=ar3_out[:], in_=ar3_in[:]))
                    mx1 = sp.tile([1, 64], F32, tag="mx1")
                    nc.sync.dma_start(out=mx1[:], in_=ar3_out[:])
                    mx_bc = sp.tile([128, 64], F32, tag="mx_bc")
                    nc.gpsimd.partition_broadcast(mx_bc[:], mx1[:])

                    nc.vector.tensor_sub(
                        attn[:].rearrange("p (t m) -> p t m", t=NT),
                        attn[:].rearrange("p (t m) -> p t m", t=NT),
                        mx_bc[:].unsqueeze(1).broadcast_to([128, NT, 64]),
                    )
                    nc.scalar.activation(attn[:], attn[:], AF.Exp)
                    nc.vector.reduce_sum(
                        red[:], attn[:].rearrange("p (t m) -> p m t", t=NT), axis=AX.X
                    )
                    nc.gpsimd.partition_all_reduce(
                        redr[:], red[:], channels=128,
                        reduce_op=bass_isa.ReduceOp.add,
                    )
                    ar4_in = dp.tile([1, 64], F32, tag="ar4_in")
                    ar4_out = dp.tile([1, 64], F32, tag="ar4_out")
                    nc.sync.dma_start(out=ar4_in[:], in_=redr[0:1, :])
                    (nc.gpsimd.collective_compute(
                        "AllReduce", OP.add, replica_groups=rg,
                        ins=[ar4_in[:]], outs=[ar4_out[:]],
                    ) if use_cc else nc.gpsimd.dma_start(out=ar4_out[:], in_=ar4_in[:]))
                    s1 = sp.tile([1, 64], F32, tag="s1")
                    nc.sync.dma_start(out=s1[:], in_=ar4_out[:])
                    nc.vector.reciprocal(s1[:], s1[:])
                    rs_bc = sp.tile([128, 64], F32, tag="rs_bc")
                    nc.gpsimd.partition_broadcast(rs_bc[:], s1[:])
                    nc.vector.tensor_mul(
                        attn[:].rearrange("p (t m) -> p t m", t=NT),
                        attn[:].rearrange("p (t m) -> p t m", t=NT),
                        rs_bc[:].unsqueeze(1).broadcast_to([128, NT, 64]),
                    )

                    wvs = atp.tile([128, NT * 256], BF16, tag="wvs")
                    nc.vector.tensor_mul(
                        wvs[:].rearrange("p (t m j) -> p t m j", t=NT, m=64),
                        attn[:].rearrange("p (t m) -> p t m", t=NT)
                        .unsqueeze(3).broadcast_to([128, NT, 64, 4]),
                        sc_bc[:].rearrange("p (m j) -> p m j", m=64)
                        .unsqueeze(1).broadcast_to([128, NT, 64, 4]),
                    )
                    nc.vector.tensor_copy(
                        wvs2[:].rearrange("p (x d) -> p x d", d=2),
                        wvs[:].unsqueeze(2).broadcast_to([128, NT * 256, 2]),
                    )
                    nc.vector.tensor_mul(
                        gtmp[:].rearrange("p (t m j) -> p t m j", t=NT, m=64),
                        attn[:].rearrange("p (t m) -> p t m", t=NT)
                        .unsqueeze(3).broadcast_to([128, NT, 64, 4]),
                        sh_bc[:].rearrange("p (m j) -> p m j", m=64)
                        .unsqueeze(1).broadcast_to([128, NT, 64, 4]),
                    )
                    nc.vector.reduce_sum(
                        w_all[:].rearrange("p (t j) -> p t j", t=NT),
                        gtmp[:].rearrange("p (t m j) -> p t j m", t=NT, m=64),
                        axis=AX.X,
                    )

            # ================= pass 3: output =================
            with (
                tc.tile_pool(name="p3raw", bufs=2) as p3r,
                tc.tile_pool(name="p3work", bufs=1) as p3w,
            ):
                for t in range(NT):
                    halfpart = p3w.tile([128, 2 * 256], F32, tag="halfpart")
                    for hf in range(2):
                        raw3 = p3r.tile([128, 8192], F32, tag="raw3")
                        nc.sync.dma_start(
                            out=raw3[:],
                            in_=spat.ap()[
                                t * 128 : t * 128 + 128,
                                hf * 8192 : hf * 8192 + 8192,
                            ],
                        )
                        raw_b = p3w.tile([128, 8192], BF16, tag="rawb3", bufs=2)
                        nc.scalar.activation(raw_b[:], raw3[:], AF.Copy)
                        if "p3c" in SKIP:
                            nc.vector.tensor_copy(
                                halfpart[:, hf * 256 : hf * 256 + 256],
                                raw_b[:, 0:256])
                            continue
                        woff = t * 512 + hf * 256
                        nc.vector.tensor_mul(
                            raw_b[:].rearrange("p (h e2 d) -> p h e2 d", h=128, e2=32),
                            raw_b[:].rearrange("p (h e2 d) -> p h e2 d", h=128, e2=32),
                            wvs2[:, woff : woff + 256]
                            .rearrange("p (h d) -> p h d", h=128)
                            .unsqueeze(2).broadcast_to([128, 128, 32, 2]),
                        )
                        # tree over m within half: m_local 32 -> 4, then reduce
                        cur, width = raw_b, 32
                        while width > 4:
                            nxt = p3w.tile(
                                [128, (width // 2) * 256], BF16, tag=f"t3_{width//2}"
                            )
                            ca = cur[:].rearrange("p (m x) -> p m x", m=width)
                            nc.vector.tensor_add(
                                nxt[:].rearrange("p (m x) -> p m x", m=width // 2),
                                ca[:, 0 : width // 2, :],
                                ca[:, width // 2 : width, :],
                            )
                            cur, width = nxt, width // 2
                        nc.vector.reduce_sum(
                            halfpart[:, hf * 256 : hf * 256 + 256]
                            .rearrange("p (j e) -> p j e", j=4),
                            cur[:].rearrange("p (m j e) -> p j e m", m=4, j=4),
                            axis=AX.X,
                        )
                    out_t = p3w.tile([128, 256], F32, tag="out_t")
                    nc.vector.tensor_add(
                        out_t[:], halfpart[:, 0:256], halfpart[:, 256:512]
                    )
                    nc.vector.tensor_add(
                        out_t[:].rearrange("p (j e) -> p j e", j=4),
                        out_t[:].rearrange("p (j e) -> p j e", j=4),
                        w_all[:, t * 4 : t * 4 + 4]
                        .unsqueeze(2).broadcast_to([128, 4, 64]),
                    )
                    nc.sync.dma_start(
                        out=out.ap()[t * 128 : t * 128 + 128, :], in_=out_t[:]
                    )
    return nc


def _prep_inputs(temp_hidden, spat_hidden, bn_gamma, bn_beta, w_temp, b_temp,
                 w_spat, b_spat, PP, n_cores):
    wq = (w_temp.T.astype(np.float64) @ w_spat.astype(np.float64)).astype(np.float32)
    wc = (w_temp.T @ b_spat).astype(np.float32)
    qb0 = (b_temp @ w_spat).astype(np.float32)
    cc0 = np.float32(b_temp @ b_spat)
    wqx = np.zeros((H, 260), np.float32)
    wqx[:, 0:H] = wq
    wqx[:, 256] = wc
    qbx = np.zeros((1, 260), np.float32)
    qbx[0, 0:H] = qb0
    qbx[0, 256] = cc0
    gb = np.stack([bn_gamma, bn_beta]).astype(np.float32)
    ident = np.eye(128, dtype=np.float32)
    ones_ = np.ones((128, 8), np.float32)

    in_maps = []
    for i in range(n_cores):
        sl = slice(i * PP, (i + 1) * PP)
        in_maps.append({
            "spat": np.ascontiguousarray(
                spat_hidden[sl].reshape(PP, H * E)).astype(np.float32),
            "temp": np.ascontiguousarray(temp_hidden[sl]).astype(np.float32),
            "wqx": wqx, "qbx": qbx, "gb": gb, "ident": ident, "ones_": ones_,
        })
    return in_maps


def kernel(temp_hidden, spat_hidden, bn_gamma, bn_beta, w_temp, b_temp,
           w_spat, b_spat):
    global _last_results
    temp_hidden = np.asarray(temp_hidden, dtype=np.float32)
    spat_hidden = np.asarray(spat_hidden, dtype=np.float32)
    P = temp_hidden.shape[0]
    PP = P // NCORES
    in_maps = _prep_inputs(
        temp_hidden, spat_hidden,
        np.asarray(bn_gamma, dtype=np.float32), np.asarray(bn_beta, dtype=np.float32),
        np.asarray(w_temp, dtype=np.float32), np.asarray(b_temp, dtype=np.float32),
        np.asarray(w_spat, dtype=np.float32), np.asarray(b_spat, dtype=np.float32),
        PP, NCORES)

    nc = bacc.Bacc("TRN2", target_bir_lowering=False, debug=False,
                   num_devices=NCORES)
    build_graph(nc, PP, NCORES, p_full=P)
    nc.compile()
    res = run_bass_kernel_spmd(nc, in_maps, core_ids=list(range(NCORES)))
    _last_results = res
    out = np.concatenate([res.results[i]["out"] for i in range(NCORES)], axis=0)
    return out.astype(np.float32)
